# revision 1
# baseline (speedup 1.0000x reference)
"""Trainium2 Bass kernel for the AudNet 4-layer LIF spiking network.

Reference computation (per time step t of 81, batch 4096):
    s1, m1 = lif(x_t @ w1.T + b1, m1)     # 129 -> 1000
    s2, m2 = lif(s1 @ w2.T + b2, m2)      # 1000 -> 1000
    s3, m3 = lif(s2 @ w3.T + b3, m3)      # 1000 -> 20
    s4, m4 = lif(s3 @ w4.T + b4, m4)      # 20 -> 10
with lif: reset = (m > 1); m' = 0.95*m + cur - reset; spk = (m' > 1)
Outputs: (s4, m4) per step -> each [81, 4096, 10].

Strategy:
- Data parallel over 8 NeuronCores: 512 batch rows per core; weights
  replicated; no cross-device traffic.
- Hidden-on-partition, batch-on-free layout: weights are the stationary
  lhsT, spikes/x the moving rhs; the 81-step scan needs no transposes.
- fp32 matmuls cost 4 PE cycles/column on TRN2, but fp32r (fp32 rounded to
  12 significand bits) costs 1. All matmuls therefore run in fp32r with a
  hi/lo split: w ~= wh + wl with wh = rne12(w), wl = rne12(w - wh), giving
  2^-24 effective weight fidelity (= fp32) at 2 cycles/column. Spikes are
  0/1 — exact in fp32r — so layers 2-4 need only (hi, lo) matmul pairs.
  Layer 1's x is also hi/lo split: wh@xh + wh@xl + wl@xh (+ a K=5 combo
  matmul carrying the last-feature row and the hi/lo bias rows).
- Biases ride the contraction dim: the last K-tile gains a ones-row whose
  lhsT entry is the (hi or lo) bias row, so they cost no extra matmul.
- LIF per layer per step is 3 elementwise passes:
    opA: m = psum + m            (DVE, reads PSUM)
    opB: spk = m > 1             (DVE is_gt, writes fp32r)
    opC: m = beta*m - spk        (fused DVE scalar_tensor_tensor, or
                                  gpsimd mul+sub for layer 1)
- Software pipelining: layer-1 psums for step t+1 run between layers 2 and
  3 of step t; layer 4 of step t runs inside iteration t+1, so the PE never
  waits on the spike-threshold chains.
"""

import os
import sys

import numpy as np

for _p in ("/opt/trn_rl_repo", "/root/.axon_site/_ro/trn_rl_repo"):
    if os.path.isdir(_p) and _p not in sys.path:
        sys.path.insert(0, _p)

import concourse.bacc as bacc
import concourse.bass as bass
import concourse.mybir as mybir
import concourse.tile as tile
from concourse.bass_utils import run_bass_kernel_spmd
from concourse.tile_rust import add_dep_helper

# Problem constants (hardcoded; kernel.py must be self-contained).
T = 81          # time steps
F = 129         # input features per step
H = 1000        # hidden units (layers 1, 2)
HT = 125        # hidden tile rows  (H = 8 * 125)
NH = 8          # number of hidden tiles
H3 = 20         # layer-3 units
H4 = 10         # output units
BATCH = 4096
NCORES = 8
B = BATCH // NCORES   # 512 batch rows per core
BETA = 0.95
THRESH = 1.0
XR = 2 * 128 + 5      # x_aug rows: xh[0:128], xl[0:128], 5 combo rows

F32 = mybir.dt.float32
F32R = mybir.dt.float32r
AOP = mybir.AluOpType


def build_bass():
    # Bacc (not raw Bass): its compile() runs generate_event_semaphores /
    # move_matmul_waits_to_ldweights, required because TRN2 Matmult
    # instructions can carry at most one sync wait.
    nc = bacc.Bacc(trn_type="TRN2", target_bir_lowering=False)

    x_d = nc.dram_tensor("x_aug", [T, XR, B], F32R, kind="ExternalInput")
    w1h_d = nc.dram_tensor("w1h", [128, H], F32R, kind="ExternalInput")
    w1l_d = nc.dram_tensor("w1l", [128, H], F32R, kind="ExternalInput")
    w1c_d = nc.dram_tensor("w1c", [5, H], F32R, kind="ExternalInput")
    w2h_d = nc.dram_tensor("w2h", [NH, HT + 1, H], F32R, kind="ExternalInput")
    w2l_d = nc.dram_tensor("w2l", [NH, HT + 1, H], F32R, kind="ExternalInput")
    # layer-3 weights as a 3-term bf16 split (8+8+8 significand bits =
    # 2^-27 fidelity); bf16 matmuls run col-tiled 4-way concurrent
    # (measured ~4.8x on HW), which fp32r cannot (walrus rejects it)
    w3b_d = nc.dram_tensor("w3b", [3 * NH, HT + 1, H3], mybir.dt.bfloat16,
                           kind="ExternalInput")
    # single layer-4 lhsT: rows 0..19 w4 hi, 20 b4 hi, 21..31 zero pad,
    # 32..51 w4 lo, 52 b4 lo — pairs with an s3 tile whose rows 32..51
    # duplicate the layer-3 spikes (partition-shift DMA)
    w4c_d = nc.dram_tensor("w4c", [53, H4], F32R, kind="ExternalInput")
    outs_d = nc.dram_tensor("out_s", [T, H4, B], F32, kind="ExternalOutput")
    outm_d = nc.dram_tensor("out_m", [T, H4, B], F32, kind="ExternalOutput")

    with tile.TileContext(nc) as tc:
        with (
            tc.tile_pool(name="pers", bufs=1) as pers,
            tc.tile_pool(name="xpool", bufs=3) as xpool,
            tc.tile_pool(name="ps1", bufs=3, space="PSUM") as ps1,
            tc.tile_pool(name="ps2", bufs=3, space="PSUM") as ps2,
            tc.tile_pool(name="ps3", bufs=1, space="PSUM") as ps3,
            tc.tile_pool(name="ps4", bufs=1, space="PSUM") as ps4,
        ):
            # ---- persistent SBUF tensors ----
            w1h = pers.tile([128, H], F32R, tag="w1h")
            w1l = pers.tile([128, H], F32R, tag="w1l")
            w1c = pers.tile([5, H], F32R, tag="w1c")
            w2h = pers.tile([HT + 1, NH * H], F32R, tag="w2h")   # [126, 8000]
            w2l = pers.tile([HT + 1, NH * H], F32R, tag="w2l")
            w3b = pers.tile([HT + 1, 3 * NH * H3], mybir.dt.bfloat16,
                            tag="w3b")                           # [126, 480]
            s2b = pers.tile([HT + 1, NH * B], mybir.dt.bfloat16, tag="s2b")
            pg3 = pers.tile([H3, 3 * B], F32, tag="pg3")  # L3 partial gather
            sg3 = pers.tile([116, B], F32, tag="sg3")     # L3 psum staging
            w4c = pers.tile([53, H4], F32R, tag="w4c")
            m1 = pers.tile([HT, NH * B], F32, tag="m1")          # [125, 4096]
            m2 = pers.tile([HT, NH * B], F32, tag="m2")
            m3 = pers.tile([H3, B], F32, tag="m3")               # [20, 512]
            m4 = pers.tile([H4, B], F32, tag="m4")               # [10, 512]
            s1 = pers.tile([HT + 1, NH * B], F32R, tag="s1")     # [126, 4096]
            s2 = pers.tile([HT + 1, NH * B], F32R, tag="s2")
            s3 = pers.tile([53, B], F32R, tag="s3")
            s4 = pers.tile([H4, B], F32, tag="s4")

            # fp32 views of the fp32r spike tiles for elementwise consumers
            s1f = s1[:].bitcast(F32)
            s2f = s2[:].bitcast(F32)
            s3f = s3[:].bitcast(F32)

            # ---- weight loads (layer-1 weights + x(0) first: they gate
            # step 0; the bulk w2/w3 transfers follow) ----
            def load_x(t):
                xh = xpool.tile([128, B], F32R, tag="xh", name="xh")
                xl = xpool.tile([128, B], F32R, tag="xl", name="xl")
                xc = xpool.tile([5, B], F32R, tag="xc", name="xc")
                nc.sync.dma_start(xh[:], x_d[t, 0:128, :])
                nc.sync.dma_start(xl[:], x_d[t, 128:256, :])
                nc.sync.dma_start(xc[:], x_d[t, 256:261, :])
                return xh, xl, xc

            w1dmas = []
            for sb, dr in [(w1h, w1h_d), (w1l, w1l_d), (w1c, w1c_d)]:
                w1dmas.append(nc.sync.dma_start(sb[:], dr[:]))
            x0 = load_x(0)
            wdmas = [nc.sync.dma_start(w4c[:], w4c_d[:])]
            for k in range(NH):
                for sb, dr, n in [(w2h, w2h_d, H), (w2l, w2l_d, H)]:
                    wdmas.append(
                        nc.sync.dma_start(sb[:, k * n:(k + 1) * n], dr[k]))
            for tk in range(3 * NH):
                wdmas.append(nc.sync.dma_start(
                    w3b[:, tk * H3:(tk + 1) * H3], w3b_d[tk]))

            # Matmult instructions can carry at most ONE sync wait in the
            # TRN2 ISA (fp32/fp32r fuse the weight load into the matmul), so
            # have PE nops absorb the weight-DMA waits before any matmul.
            # Layer-1 absorbers go before the prologue; the rest only need to
            # precede the main loop's first layer-2/3/4 matmuls.
            def absorb(dmas):
                nops = []
                for d in dmas:
                    nop = nc.tensor.nop(nofuse=True)
                    add_dep_helper(nop.ins, d.ins, sync=True,
                                   reason="absorb weight-DMA wait on PE")
                    nops.append(nop)
                return nops

            absorbers = absorb(w1dmas)

            # ---- state init ----
            nc.vector.memset(m1[:], 0.0)
            nc.vector.memset(m2[:], 0.0)
            nc.gpsimd.memset(m3[:], 0.0)
            nc.gpsimd.memset(m4[:], 0.0)
            # ones rows feeding the bias fold (k-tile 7 / layer-4 rhs).
            # Engine ops need partition bases in {0,32,64,96}, so memset a
            # wider aligned region; all rows except the last are overwritten
            # by the per-step spike writes before any matmul reads them.
            nc.vector.memset(s1f[96:HT + 1, (NH - 1) * B:], 1.0)
            nc.vector.memset(s2f[96:HT + 1, (NH - 1) * B:], 1.0)
            nc.gpsimd.memset(s3f[:, :], 1.0)   # rows 20/52 stay as ones rows

            def l1_block(xh, xl, xc):
                """Layer-1 psums + LIF opA/opB for one step, per hidden tile."""
                first_mm = None
                for h in range(NH):
                    p1 = ps1.tile([HT, B], F32, tag="p1")
                    c0 = h * HT
                    mm = nc.tensor.matmul(p1[:], w1h[:, c0:c0 + HT], xh[:],
                                          start=True, stop=False)
                    if first_mm is None:
                        first_mm = mm
                    nc.tensor.matmul(p1[:], w1h[:, c0:c0 + HT], xl[:],
                                     start=False, stop=False)
                    nc.tensor.matmul(p1[:], w1l[:, c0:c0 + HT], xh[:],
                                     start=False, stop=False)
                    nc.tensor.matmul(p1[:], w1c[:, c0:c0 + HT], xc[:],
                                     start=False, stop=True)
                    cols = slice(h * B, (h + 1) * B)
                    nc.vector.tensor_tensor(m1[:, cols], p1[:], m1[:, cols],
                                            AOP.add)
                    nc.vector.tensor_scalar(s1[0:HT, cols], m1[:, cols],
                                            THRESH, None, AOP.is_gt)
                return first_mm

            def l1_state_update():
                # m1 = beta*m1 - spk1 on gpsimd (no scalar_tensor_tensor
                # there), two in-place ops
                nc.gpsimd.tensor_scalar_mul(m1[:], m1[:], BETA)
                nc.gpsimd.tensor_tensor(m1[:], m1[:], s1f[0:HT, :],
                                        AOP.subtract)

            # ---- prologue: step 0 layer-1 ----
            first_mm = l1_block(*x0)
            for nop in absorbers:
                add_dep_helper(first_mm.ins, nop.ins, sync=False,
                               reason="keep absorbers before first matmul")
            l1_state_update()

            # absorb the remaining weight DMAs before the main loop's
            # layer-2/3/4 matmuls
            late_absorbers = absorb(wdmas)

            def l4_block(t):
                """Layer 4 for step t + LIF + output DMAs."""
                p4 = ps4.tile([H4, B], F32, tag="p4")
                nc.tensor.matmul(p4[:], w4c[:], s3[:], start=True, stop=True)
                nc.vector.tensor_tensor(m4[:], p4[:], m4[:], AOP.add)
                nc.sync.dma_start(outm_d[t], m4[:])
                nc.gpsimd.tensor_scalar(s4[:], m4[:], THRESH, None, AOP.is_gt)
                nc.sync.dma_start(outs_d[t], s4[:])
                nc.vector.scalar_tensor_tensor(m4[:], m4[:], BETA, s4[:],
                                               AOP.mult, AOP.subtract)

            # ---- main loop over steps ----
            for i in range(T):
                if i < T - 1:
                    xh, xl, xc = load_x(i + 1)

                # layer 2 of step i
                for h in range(NH):
                    p2 = ps2.tile([HT, B], F32, tag="p2")
                    c0 = h * HT
                    for k in range(NH):
                        kk = HT + 1 if k == NH - 1 else HT
                        for wt in (w2h, w2l):
                            mm2 = nc.tensor.matmul(
                                p2[:],
                                wt[0:kk, k * H + c0:k * H + c0 + HT],
                                s1[0:kk, k * B:(k + 1) * B],
                                start=(k == 0 and wt is w2h),
                                stop=(k == NH - 1 and wt is w2l))
                            if i == 0 and h == 0 and k == 0 and wt is w2h:
                                for nop in late_absorbers:
                                    add_dep_helper(
                                        mm2.ins, nop.ins, sync=False,
                                        reason="absorbers before first L2 mm")
                    cols = slice(h * B, (h + 1) * B)
                    nc.vector.tensor_tensor(m2[:, cols], p2[:], m2[:, cols],
                                            AOP.add)
                    nc.vector.tensor_scalar(s2[0:HT, cols], m2[:, cols],
                                            THRESH, None, AOP.is_gt)

                # layer 4 of step i-1 (deferred so spk3 is long ready)
                if i > 0:
                    l4_block(i - 1)

                # layer-1 psums + LIF for step i+1
                if i < T - 1:
                    l1_block(xh, xl, xc)

                # layer-2 state update (off critical path)
                nc.vector.scalar_tensor_tensor(m2[:], m2[:], BETA,
                                               s2f[0:HT, :],
                                               AOP.mult, AOP.subtract)

                # layer 3 of step i: bf16 spikes (0/1 exact) x 3-term bf16
                # weights, 4 K-chunk groups running concurrently on separate
                # 32-column PE groups; partials gathered via ACT copy + DMA
                # partition shifts (engines are lane-locked)
                nc.vector.tensor_copy(s2b[:], s2f[:])
                p3 = ps3.tile([128, B], F32, tag="p3")
                for g in range(4):
                    first = True
                    for k in (2 * g, 2 * g + 1):
                        kk = HT + 1 if k == NH - 1 else HT
                        for t in range(3):
                            nc.tensor.matmul(
                                p3[32 * g:32 * g + H3, :],
                                w3b[0:kk, (t * NH + k) * H3:(t * NH + k + 1) * H3],
                                s2b[0:kk, k * B:(k + 1) * B],
                                start=first,
                                stop=(k == 2 * g + 1 and t == 2),
                                tile_position=(0, 32 * g))
                            first = False
                nc.scalar.copy(sg3[:], p3[0:116, :])
                for g in range(1, 4):
                    nc.sync.dma_start(pg3[:, (g - 1) * B:g * B],
                                      sg3[32 * g:32 * g + H3, :])
                nc.vector.tensor_tensor(m3[:], sg3[0:H3, :], m3[:], AOP.add)
                for g in range(3):
                    nc.vector.tensor_tensor(m3[:], pg3[:, g * B:(g + 1) * B],
                                            m3[:], AOP.add)
                nc.gpsimd.tensor_scalar(s3[0:H3, :], m3[:], THRESH, None,
                                        AOP.is_gt)
                # duplicate layer-3 spikes into rows 32..51 for the packed
                # hi+lo layer-4 matmul (engines are lane-locked; DMA shifts
                # partitions)
                nc.sync.dma_start(s3[32:32 + H3, :], s3[0:H3, :])
                nc.vector.scalar_tensor_tensor(m3[:], m3[:], BETA,
                                               s3f[0:H3, :],
                                               AOP.mult, AOP.subtract)

                # layer-1 state update for step i+1
                if i < T - 1:
                    l1_state_update()

            # ---- epilogue ----
            l4_block(T - 1)

    nc.compile()
    return nc


_CACHE = {}


def _get_nc():
    if "nc" not in _CACHE:
        _CACHE["nc"] = build_bass()
    return _CACHE["nc"]


def _rne12(a):
    """Round fp32 to 12 significand bits (the fp32r grid), RNE —
    bit-identical to the device's fp32r rounding."""
    drop = np.uint64(12)
    u = np.ascontiguousarray(a, np.float32).view(np.uint32).astype(np.uint64)
    half = np.uint64(1 << 11)
    lsb = (u >> drop) & np.uint64(1)
    u2 = ((u + half - np.uint64(1) + lsb) >> drop << drop)
    return u2.astype(np.uint32).view(np.float32).reshape(a.shape)


def _hilo(a):
    hi = _rne12(a)
    lo = _rne12(np.asarray(a, np.float32) - hi)
    return hi, lo


def _prep_inputs(x, w1, b1, w2, b2, w3, b3, w4, b4):
    x = np.ascontiguousarray(x, np.float32)
    # xs[t, f, b_global]; step t of the reference reads x[:, f*T + t]
    xt = np.ascontiguousarray(
        np.transpose(x.reshape(BATCH, F, T), (2, 1, 0)))   # [T, F, BATCH]
    xth, xtl = _hilo(xt)

    w1T = np.ascontiguousarray(w1.T.astype(np.float32))    # [129, 1000]
    w1h, w1l = _hilo(w1T[:128])
    whL, wlL = _hilo(w1T[128])
    b1h, b1l = _hilo(b1.astype(np.float32))
    w1c = np.stack([whL, whL, wlL, b1h, b1l])              # [5, 1000]

    def kaug(wT, bias, nout):                              # [NH, HT+1, nout]
        h_, l_ = _hilo(wT)
        bh, bl = _hilo(bias.astype(np.float32))
        oh = np.zeros((NH, HT + 1, nout), np.float32)
        ol = np.zeros((NH, HT + 1, nout), np.float32)
        for k in range(NH):
            oh[k, :HT] = h_[k * HT:(k + 1) * HT]
            ol[k, :HT] = l_[k * HT:(k + 1) * HT]
        oh[NH - 1, HT] = bh
        ol[NH - 1, HT] = bl
        return oh, ol

    w2h, w2l = kaug(np.ascontiguousarray(w2.T.astype(np.float32)), b2, H)

    # layer-3: 3-term bf16 split of weights and bias
    import ml_dtypes
    bf16 = ml_dtypes.bfloat16

    def bf16_terms(a):
        a = np.asarray(a, np.float32)
        t1 = a.astype(bf16)
        t2 = (a - t1.astype(np.float32)).astype(bf16)
        t3 = (a - t1.astype(np.float32) - t2.astype(np.float32)).astype(bf16)
        return t1, t2, t3

    w3terms = bf16_terms(w3.T.astype(np.float32))          # each [1000, 20]
    b3terms = bf16_terms(b3.astype(np.float32))
    w3b = np.zeros((3 * NH, HT + 1, H3), bf16)
    for t in range(3):
        for k in range(NH):
            w3b[t * NH + k, :HT] = w3terms[t][k * HT:(k + 1) * HT]
        w3b[t * NH + NH - 1, HT] = b3terms[t]
    w4T = w4.T.astype(np.float32)                          # [20, 10]
    w4hh, w4ll = _hilo(w4T)
    b4h, b4l = _hilo(b4.astype(np.float32))
    w4c = np.zeros((53, H4), np.float32)
    w4c[0:20] = w4hh
    w4c[20] = b4h
    w4c[32:52] = w4ll
    w4c[52] = b4l

    in_maps = []
    for c in range(NCORES):
        xc = np.empty((T, XR, B), np.float32)
        xc[:, 0:128, :] = xth[:, 0:128, c * B:(c + 1) * B]
        xc[:, 128:256, :] = xtl[:, 0:128, c * B:(c + 1) * B]
        xc[:, 256, :] = xth[:, 128, c * B:(c + 1) * B]
        xc[:, 257, :] = xtl[:, 128, c * B:(c + 1) * B]
        xc[:, 258, :] = xth[:, 128, c * B:(c + 1) * B]
        xc[:, 259, :] = 1.0
        xc[:, 260, :] = 1.0
        in_maps.append({
            "x_aug": xc, "w1h": w1h, "w1l": w1l, "w1c": w1c,
            "w2h": w2h, "w2l": w2l, "w3b": w3b, "w4c": w4c,
        })
    return in_maps


def _gather(results):
    spk = np.concatenate(
        [np.transpose(r["out_s"], (0, 2, 1)) for r in results], axis=1)
    mem = np.concatenate(
        [np.transpose(r["out_m"], (0, 2, 1)) for r in results], axis=1)
    return spk, mem


def kernel(x, w1, b1, w2, b2, w3, b3, w4, b4, _trace=False, _trace_kwargs=None):
    # accept numpy or jax arrays, any float dtype
    x, w1, b1, w2, b2, w3, b3, w4, b4 = (
        np.asarray(a, dtype=np.float32)
        for a in (x, w1, b1, w2, b2, w3, b3, w4, b4))
    nc = _get_nc()
    in_maps = _prep_inputs(x, w1, b1, w2, b2, w3, b3, w4, b4)
    res = run_bass_kernel_spmd(
        nc, in_maps, core_ids=list(range(NCORES)),
        trace=_trace, **(_trace_kwargs or {}))
    out = _gather(res.results)
    if _trace:
        return out, res
    return out



# revision 10
# speedup vs baseline: 1.7407x; 1.7407x over previous
"""Trainium2 Bass kernel for the AudNet 4-layer LIF spiking network.

Reference computation (per time step t of 81, batch 4096):
    s1, m1 = lif(x_t @ w1.T + b1, m1)     # 129 -> 1000
    s2, m2 = lif(s1 @ w2.T + b2, m2)      # 1000 -> 1000
    s3, m3 = lif(s2 @ w3.T + b3, m3)      # 1000 -> 20
    s4, m4 = lif(s3 @ w4.T + b4, m4)      # 20 -> 10
with lif: reset = (m > 1); m' = 0.95*m + cur - reset; spk = (m' > 1)
Outputs: (s4, m4) per step -> each [81, 4096, 10].

Strategy (v3):
- Data parallel over 8 NeuronCores: 512 batch rows per core; weights
  replicated; no cross-device traffic.
- Hidden layers padded 1000 -> 1024 (8 tiles x 128): fp8 DoubleRow
  matmuls require full 32/64/128 weight-tile columns.
- Layers 1-3 run in a scaled membrane domain M = 2^12 * m so e4m3
  weight terms (w*2^12) are in-range, the sigmoid threshold trick is
  fp32-exact, and the LIF update is ONE fused DVE op per psum tile:
  M = beta*M + psum. The psum carries current, bias AND the
  -2^12*s_prev reset (via a -2048*I e5m2 diagonal, self-paired in
  DoubleRow: equal exponents make the 11-bit pair-combine exact).
- L2/L3 weights: 3-term e4m3 "band split" of w*2^12 run as DR matmuls
  at 0.5 PE cycles/col. The measured HW DR pair-combine is an 11-bit
  RNE adder, so term A holds only the >=4-magnitude entries (pair sums
  fit 11 bits: exact); terms B (<=4) and C (<=0.125) have negligible
  absolute combine error. Verified 0 output spike flips in a full-
  input numpy model of exactly this arithmetic.
- Spikes {0,1} live in e5m2 (fp8 matmul rhs), written by ACT as
  sigmoid(2^18*(M - 2^12)) which saturates to exactly 0.0/1.0.
- Biases ride always-on pad neurons: L1 pad units 1000..1007 get a
  +2^13 fp32r bias (combo matmul) so they fire every step and act as
  the ones-rows for L2's bias row; L2's pad unit 1000 is pulled up by
  3x8x224 e4m3 entries and provides L3's ones-row.
- L1 keeps the fp32r hi/lo split of x and w1 (24-bit; anything less
  flips thousands of output spikes) scaled by 2^12, 4 matmuls/tile,
  plus the DR diag.
- L4 is one packed fp32r matmul: lhsT rows [w4h|0|w4l|0|-I|b4h|b4l]
  vs rhs rows [s3|0|s3dup|0|s4_prev|ones]: bias, hi/lo split and the
  m4 reset ride one 213ns instruction. m4 stays unscaled (it is the
  output); s3/s4 are fp32r {0,1} written by Pool is_gt.
- Software pipeline per iteration i: L1(i+1)+ACT s1 (double-buffered);
  L2(i); L4(i-2); L3(i-1); ACT s2(i) last. The deferred L3/L4 keep the
  PE from ever waiting on a sigmoid.
"""

import os
import sys

import numpy as np
import ml_dtypes

for _p in ("/opt/trn_rl_repo", "/root/.axon_site/_ro/trn_rl_repo"):
    if os.path.isdir(_p) and _p not in sys.path:
        sys.path.insert(0, _p)

import concourse.bacc as bacc
import concourse.mybir as mybir
import concourse.tile as tile
from concourse.bass_utils import run_bass_kernel_spmd
from concourse.tile_rust import add_dep_helper

# Problem constants (hardcoded; kernel.py must be self-contained).
T = 81          # time steps
F = 129         # input features per step
H = 1000        # real hidden units (layers 1, 2)
HT = 128        # hidden tile rows (padded)
NH = 8          # number of hidden tiles
HP = NH * HT    # padded hidden 1024
H3 = 20         # layer-3 units
H3P = 32        # padded layer-3 psum partitions
H4 = 10         # output units
BATCH = 4096
NCORES = 8
B = BATCH // NCORES   # 512 batch rows per core
BETA = 0.95
XR = 2 * 128 + 5      # x_aug rows: xh[0:128], xl[0:128], 5 combo rows
MS = 4096.0           # membrane scale 2^12
SIG_SCALE = float(2.0 ** 18)
SIG_BIAS = float(-(2.0 ** 30))
K4 = 76               # L4 packed contraction rows
NONES = 8             # always-on L1 pad units 1000..1007

E4NP = ml_dtypes.float8_e4m3
E5NP = ml_dtypes.float8_e5m2

F32 = mybir.dt.float32
F32R = mybir.dt.float32r
FE4 = mybir.dt.float8e4
FE5 = mybir.dt.float8e5
AOP = mybir.AluOpType
DR = mybir.MatmulPerfMode.DoubleRow
SIGMOID = mybir.ActivationFunctionType.Sigmoid


def build_bass():
    # Bacc (not raw Bass): its compile() runs generate_event_semaphores /
    # move_matmul_waits_to_ldweights, required because TRN2 Matmult
    # instructions can carry at most one sync wait.
    nc = bacc.Bacc(trn_type="TRN2", target_bir_lowering=False)

    x_d = nc.dram_tensor("x_aug", [T, XR, B], F32R, kind="ExternalInput")
    w1h_d = nc.dram_tensor("w1h", [128, HP], F32R, kind="ExternalInput")
    w1l_d = nc.dram_tensor("w1l", [128, HP], F32R, kind="ExternalInput")
    w1c_d = nc.dram_tensor("w1c", [5, HP], F32R, kind="ExternalInput")
    w2t_d = [nc.dram_tensor(f"w2t{i}", [HT, NH, HP], FE4,
                            kind="ExternalInput") for i in range(3)]
    w3t_d = [nc.dram_tensor(f"w3t{i}", [HT, NH, H3P], FE4,
                            kind="ExternalInput") for i in range(3)]
    dg_d = nc.dram_tensor("dg", [HT, 2, HT], FE5, kind="ExternalInput")
    dg3_d = nc.dram_tensor("dg3", [H3, H3], F32R, kind="ExternalInput")
    w4c_d = nc.dram_tensor("w4c", [K4, H4], F32R, kind="ExternalInput")
    ones_d = nc.dram_tensor("ones2", [2, B], F32R, kind="ExternalInput")
    outs_d = nc.dram_tensor("out_s", [T, H4, B], F32, kind="ExternalOutput")
    outm_d = nc.dram_tensor("out_m", [T, H4, B], F32, kind="ExternalOutput")

    with tile.TileContext(nc) as tc:
        with (
            tc.tile_pool(name="pers", bufs=1) as pers,
            tc.tile_pool(name="xpool", bufs=3) as xpool,
            tc.tile_pool(name="ps1", bufs=3, space="PSUM") as ps1,
            tc.tile_pool(name="ps2", bufs=3, space="PSUM") as ps2,
            tc.tile_pool(name="ps3", bufs=1, space="PSUM") as ps3,
            tc.tile_pool(name="ps4", bufs=1, space="PSUM") as ps4,
        ):
            # ---- persistent SBUF tensors ----
            w1h = pers.tile([128, HP], F32R, tag="w1h")
            w1l = pers.tile([128, HP], F32R, tag="w1l")
            w1c = pers.tile([5, HP], F32R, tag="w1c")
            w2t = [pers.tile([HT, NH, HP], FE4, tag=f"w2t{i}",
                             name=f"w2t{i}") for i in range(3)]
            w3t = [pers.tile([HT, NH, H3P], FE4, tag=f"w3t{i}",
                             name=f"w3t{i}") for i in range(3)]
            dg = pers.tile([HT, 2, HT], FE5, tag="dg")      # -2048*I twice
            dg3 = pers.tile([H3, H3], F32R, tag="dg3")      # -4096*I
            w4c = pers.tile([K4, H4], F32R, tag="w4c")
            bias_t = pers.tile([128, 1], F32, tag="bias_t")  # -2^30

            M1 = pers.tile([HT, NH, B], F32, tag="M1")      # 2^12 * m1
            M2 = pers.tile([HT, NH, B], F32, tag="M2")
            M3 = pers.tile([H3P, B], F32, tag="M3")         # 2^12 * m3
            m4 = pers.tile([H4, B], F32, tag="m4")          # unscaled
            s1b = [pers.tile([HT, NH, B], FE5, tag=f"s1_{j}",
                             name=f"s1_{j}") for j in range(2)]  # dbl buf
            s2 = pers.tile([HT, NH, B], FE5, tag="s2")
            s3t = pers.tile([K4, B], F32R, tag="s3t")
            s3f = s3t[:].bitcast(F32)

            # ---- weight loads (layer-1 + x(0) + diag first: they gate
            # step 0; the bulk w2/w3 transfers follow) ----
            def load_x(t):
                xh = xpool.tile([128, B], F32R, tag="xh", name="xh")
                xl = xpool.tile([128, B], F32R, tag="xl", name="xl")
                xc = xpool.tile([5, B], F32R, tag="xc", name="xc")
                nc.sync.dma_start(xh[:], x_d[t, 0:128, :])
                nc.sync.dma_start(xl[:], x_d[t, 128:256, :])
                nc.sync.dma_start(xc[:], x_d[t, 256:261, :])
                return xh, xl, xc

            w1dmas = []
            for sb, dr_ in [(w1h, w1h_d), (w1l, w1l_d), (w1c, w1c_d),
                            (dg, dg_d)]:
                w1dmas.append(nc.sync.dma_start(sb[:], dr_[:]))
            x0 = load_x(0)
            wdmas = [nc.sync.dma_start(w4c[:], w4c_d[:]),
                     nc.sync.dma_start(dg3[:], dg3_d[:]),
                     nc.sync.dma_start(s3t[74:76, :], ones_d[:])]
            for i in range(3):
                wdmas.append(nc.sync.dma_start(w2t[i][:], w2t_d[i][:]))
                wdmas.append(nc.sync.dma_start(w3t[i][:], w3t_d[i][:]))

            # Matmult instructions can carry at most ONE sync wait in the
            # TRN2 ISA, so have PE nops absorb the weight-DMA waits before
            # any matmul.
            def absorb(dmas):
                nops = []
                for d in dmas:
                    nop = nc.tensor.nop(nofuse=True)
                    add_dep_helper(nop.ins, d.ins, sync=True,
                                   reason="absorb weight-DMA wait on PE")
                    nops.append(nop)
                return nops

            absorbers = absorb(w1dmas)

            # ---- state init ----
            nc.vector.memset(bias_t[:], SIG_BIAS)
            nc.vector.memset(M1[:], 0.0)
            nc.vector.memset(M2[:], 0.0)
            nc.gpsimd.memset(M3[:], 0.0)
            nc.gpsimd.memset(m4[:], 0.0)
            for j in range(2):
                nc.gpsimd.memset(s1b[j][:].bitcast(mybir.dt.uint8), 0)
            nc.gpsimd.memset(s2[:].bitcast(mybir.dt.uint8), 0)
            # s3t rows 0..73 zero; rows 74/75 are DMA'd ones.
            nc.gpsimd.memset(s3t[0:64].bitcast(F32), 0.0)
            nc.gpsimd.memset(s3t[64:74].bitcast(F32), 0.0)

            def sig(out_sl, in_sl, np_):
                return nc.scalar.activation(out_sl, in_sl, SIGMOID,
                                            bias=bias_t[0:np_],
                                            scale=SIG_SCALE)

            def l1_block(xh, xl, xc, sprev, first_abs=None):
                """Layer-1 psums + M1 update for one step.

                psum = 2^12*(w1 x + b1) - 2^12*s1_prev; M1 = b*M1 + psum.
                """
                first_mm = None
                for h in range(NH):
                    p1 = ps1.tile([HT, B], F32, tag="p1")
                    c0 = h * HT
                    mm = nc.tensor.matmul(p1[:], w1h[:, c0:c0 + HT], xh[:],
                                          start=True, stop=False)
                    if first_mm is None:
                        first_mm = mm
                        if first_abs:
                            for nop in first_abs:
                                add_dep_helper(
                                    mm.ins, nop.ins, sync=False,
                                    reason="absorbers before first matmul")
                    nc.tensor.matmul(p1[:], w1h[:, c0:c0 + HT], xl[:],
                                     start=False, stop=False)
                    nc.tensor.matmul(p1[:], w1l[:, c0:c0 + HT], xh[:],
                                     start=False, stop=False)
                    nc.tensor.matmul(p1[:], w1c[:, c0:c0 + HT], xc[:],
                                     start=False, stop=False)
                    nc.tensor.matmul(
                        p1[:], dg[:],
                        sprev[:, h:h + 1, :].to_broadcast((HT, 2, B)),
                        start=False, stop=True, perf_mode=DR)
                    nc.vector.scalar_tensor_tensor(
                        M1[:, h, :], M1[:, h, :], BETA, p1[:],
                        AOP.mult, AOP.add)

            # ---- prologue: step 0 layer-1 + s1(0) ----
            l1_block(*x0, sprev=s1b[1], first_abs=absorbers)
            sig(s1b[0][:], M1[:], HT)

            late_absorbers = absorb(wdmas)

            def l4_block(t):
                """Layer 4 for step t: packed matmul + LIF + output DMAs."""
                p4 = ps4.tile([H4, B], F32, tag="p4")
                nc.tensor.matmul(p4[:], w4c[:], s3t[:], start=True, stop=True)
                nc.vector.scalar_tensor_tensor(m4[:], m4[:], BETA, p4[:],
                                               AOP.mult, AOP.add)
                nc.sync.dma_start(outm_d[t], m4[:])
                nc.gpsimd.tensor_scalar(s3t[64:74, :], m4[:], 1.0, None,
                                        AOP.is_gt)
                nc.sync.dma_start(outs_d[t], s3f[64:74, :])

            def l3_block():
                """Layer 3: 12 DR band matmuls + fp32r reset diag."""
                p3 = ps3.tile([H3P, B], F32, tag="p3")
                first = True
                for ti in range(3):
                    for g in range(4):
                        nc.tensor.matmul(
                            p3[:],
                            w3t[ti][:, 2 * g:2 * g + 2, :],
                            s2[:, 2 * g:2 * g + 2, :],
                            start=first, stop=False, perf_mode=DR)
                        first = False
                nc.tensor.matmul(p3[0:H3, :], dg3[:], s3t[0:H3, :],
                                 start=False, stop=True)
                nc.vector.scalar_tensor_tensor(M3[:], M3[:], BETA, p3[:],
                                               AOP.mult, AOP.add)
                nc.gpsimd.tensor_scalar(s3t[0:H3, :], M3[0:H3, :], MS, None,
                                        AOP.is_gt)
                # duplicate layer-3 spikes into rows 32..51 for the packed
                # hi+lo layer-4 matmul (DMA shifts partitions)
                nc.sync.dma_start(s3t[32:32 + H3, :], s3t[0:H3, :])

            # ---- main loop over steps ----
            for i in range(T):
                scur = s1b[i % 2]        # s1(i)
                snxt = s1b[(i + 1) % 2]  # s1(i+1)

                # layer 1 of step i+1 (+ its M1 update and sigmoid)
                if i < T - 1:
                    xs = load_x(i + 1)
                    l1_block(*xs, sprev=scur)
                    sig(snxt[:], M1[:], HT)

                # layer 2 of step i
                for h in range(NH):
                    p2 = ps2.tile([HT, B], F32, tag="p2")
                    c0 = h * HT
                    first_mm2 = None
                    for ti in range(3):
                        for g in range(4):
                            mm2 = nc.tensor.matmul(
                                p2[:],
                                w2t[ti][:, 2 * g:2 * g + 2, c0:c0 + HT],
                                scur[:, 2 * g:2 * g + 2, :],
                                start=(ti == 0 and g == 0), stop=False,
                                perf_mode=DR)
                            if first_mm2 is None:
                                first_mm2 = mm2
                                if i == 0 and h == 0:
                                    for nop in late_absorbers:
                                        add_dep_helper(
                                            mm2.ins, nop.ins, sync=False,
                                            reason="absorbers before L2")
                    nc.tensor.matmul(
                        p2[:], dg[:],
                        s2[:, h:h + 1, :].to_broadcast((HT, 2, B)),
                        start=False, stop=True, perf_mode=DR)
                    nc.vector.scalar_tensor_tensor(
                        M2[:, h, :], M2[:, h, :], BETA, p2[:],
                        AOP.mult, AOP.add)

                # layer 4 of step i-2 (deferred; s3(i-2) long ready)
                if i >= 2:
                    l4_block(i - 2)

                # layer 3 of step i-1 (deferred; s2(i-1) long ready; must
                # read s2 BEFORE this step's sigmoid overwrites it)
                if i >= 1:
                    l3_block()

                # s2(i) = sigmoid(M2): after L2 stts and after L3(i-1)
                # finished reading s2(i-1)
                sig(s2[:], M2[:], HT)

            # ---- epilogue ----
            l4_block(T - 2)
            l3_block()
            l4_block(T - 1)

    nc.compile()
    return nc


_CACHE = {}


def _get_nc():
    if "nc" not in _CACHE:
        _CACHE["nc"] = build_bass()
    return _CACHE["nc"]


def _rne12(a):
    """Round fp32 to 12 significand bits (the fp32r grid), RNE."""
    drop = np.uint64(12)
    u = np.ascontiguousarray(a, np.float32).view(np.uint32).astype(np.uint64)
    half = np.uint64(1 << 11)
    lsb = (u >> drop) & np.uint64(1)
    u2 = ((u + half - np.uint64(1) + lsb) >> drop << drop)
    return u2.astype(np.uint32).view(np.float32).reshape(a.shape)


def _hilo(a):
    hi = _rne12(np.asarray(a, np.float32))
    lo = _rne12((np.asarray(a, np.float64) - hi).astype(np.float32))
    return hi, lo


def _band3(w64):
    """3-term e4m3 band split of a 2^12-scaled fp64 array.

    t_A: entries of e4m3(w) with |v| >= 4 (DR pair sums exact at 11 bits)
    t_B: e4m3(residual)  (|v| <= 4: combine error <= 2^-9 absolute)
    t_C: e4m3(residual2) (|v| <= 0.125)
    """
    t1 = w64.astype(np.float32).astype(E4NP).astype(np.float64)
    ta = np.where(np.abs(t1) >= 4.0, t1, 0.0)
    r = w64 - ta
    tb = r.astype(np.float32).astype(E4NP)
    r2 = r - tb.astype(np.float64)
    tc = r2.astype(np.float32).astype(E4NP)
    return ta.astype(np.float32), tb.astype(np.float32), tc.astype(np.float32)


def _pack_terms(wT, bias, pullup_col=None):
    """[HT, NH, nout] e4m3 term tiles for a padded (1024 -> nout) layer.

    wT: [1000, nout] real weights (already col-padded); bias rides
    padded in-row 1000 (the first always-on pad neuron of the previous
    layer); optional pull-up column gets 3x8x224 from in-rows
    1000..1007.
    """
    nout = wT.shape[1]
    w64 = np.zeros((HP, nout), np.float64)
    w64[0:H] = np.asarray(wT, np.float64) * MS
    w64[H] += np.asarray(bias, np.float64) * MS
    terms = _band3(w64)
    out = []
    for t_ in terms:
        if pullup_col is not None:
            t_[H:H + NONES, pullup_col] = 224.0
        arr = np.zeros((HT, NH, nout), E4NP)
        for k in range(NH):
            arr[:, k, :] = t_[k * HT:(k + 1) * HT, :]
        out.append(arr)
    return out


def _prep_inputs(x, w1, b1, w2, b2, w3, b3, w4, b4):
    x = np.ascontiguousarray(x, np.float32)
    # xs[t, f, b_global]; step t of the reference reads x[:, f*T + t]
    xt = np.ascontiguousarray(
        np.transpose(x.reshape(BATCH, F, T), (2, 1, 0)))   # [T, F, BATCH]
    xth, xtl = _hilo(xt)

    # L1 weights: [129, 1024] padded, scaled by 2^12.
    w1P = np.zeros((F, HP), np.float64)
    w1P[:, 0:H] = np.asarray(w1, np.float64).T * MS
    w1h, w1l = _hilo(w1P[0:128].astype(np.float32))
    whL, wlL = _hilo(w1P[128].astype(np.float32))
    b1r = np.zeros(HP, np.float64)
    b1r[0:H] = np.asarray(b1, np.float64) * MS
    b1r[H:H + NONES] = 2.0 ** 13        # always-on pad-neuron pull-up
    b1h, b1l = _hilo(b1r.astype(np.float32))
    w1c = np.stack([whL, whL, wlL, b1h, b1l])              # [5, 1024]

    # L2: cols padded to 1024; pull-up col = padded unit 1000.
    w2full = np.zeros((H, HP), np.float32)
    w2full[:, 0:H] = np.asarray(w2, np.float32).T
    b2full = np.zeros(HP, np.float32)
    b2full[0:H] = np.asarray(b2, np.float32)
    w2terms = _pack_terms(w2full, b2full, pullup_col=H)
    w3full = np.zeros((H, H3P), np.float32)
    w3full[:, 0:H3] = np.asarray(w3, np.float32).T
    b3full = np.zeros(H3P, np.float32)
    b3full[0:H3] = np.asarray(b3, np.float32)
    w3terms = _pack_terms(w3full, b3full)

    dgv = np.zeros((HT, 2, HT), np.float32)
    for i in range(HT):
        dgv[i, 0, i] = -2048.0
        dgv[i, 1, i] = -2048.0
    dg = dgv.astype(E5NP)
    dg3 = (-MS * np.eye(H3, dtype=np.float32))

    # L4 packed lhsT: rows 0..19 w4h, 32..51 w4l, 64..73 -I (s4 reset),
    # 74 b4h, 75 b4l; rest zero.
    w4T = np.asarray(w4, np.float32).T                     # [20, 10]
    w4h, w4l = _hilo(w4T)
    b4h, b4l = _hilo(np.asarray(b4, np.float32))
    w4c = np.zeros((K4, H4), np.float32)
    w4c[0:H3] = w4h
    w4c[32:32 + H3] = w4l
    w4c[64:64 + H4] = -np.eye(H4, dtype=np.float32)
    w4c[74] = b4h
    w4c[75] = b4l

    ones2 = np.ones((2, B), np.float32)

    in_maps = []
    for c in range(NCORES):
        xc = np.empty((T, XR, B), np.float32)
        xc[:, 0:128, :] = xth[:, 0:128, c * B:(c + 1) * B]
        xc[:, 128:256, :] = xtl[:, 0:128, c * B:(c + 1) * B]
        xc[:, 256, :] = xth[:, 128, c * B:(c + 1) * B]
        xc[:, 257, :] = xtl[:, 128, c * B:(c + 1) * B]
        xc[:, 258, :] = xth[:, 128, c * B:(c + 1) * B]
        xc[:, 259, :] = 1.0
        xc[:, 260, :] = 1.0
        m = {"x_aug": xc, "w1h": w1h, "w1l": w1l, "w1c": w1c,
             "dg": dg, "dg3": dg3, "w4c": w4c, "ones2": ones2}
        for i in range(3):
            m[f"w2t{i}"] = w2terms[i]
            m[f"w3t{i}"] = w3terms[i]
        in_maps.append(m)
    return in_maps


def _gather(results):
    spk = np.concatenate(
        [np.transpose(r["out_s"], (0, 2, 1)) for r in results], axis=1)
    mem = np.concatenate(
        [np.transpose(r["out_m"], (0, 2, 1)) for r in results], axis=1)
    return spk, mem


def kernel(x, w1, b1, w2, b2, w3, b3, w4, b4, _trace=False, _trace_kwargs=None):
    x, w1, b1, w2, b2, w3, b3, w4, b4 = (
        np.asarray(a, dtype=np.float32)
        for a in (x, w1, b1, w2, b2, w3, b3, w4, b4))
    nc = _get_nc()
    in_maps = _prep_inputs(x, w1, b1, w2, b2, w3, b3, w4, b4)
    res = run_bass_kernel_spmd(
        nc, in_maps, core_ids=list(range(NCORES)),
        trace=_trace, **(_trace_kwargs or {}))
    out = _gather(res.results)
    if _trace:
        return out, res
    return out


# revision 11
# speedup vs baseline: 1.7771x; 1.0209x over previous
"""Trainium2 Bass kernel for the AudNet 4-layer LIF spiking network.

Reference computation (per time step t of 81, batch 4096):
    s1, m1 = lif(x_t @ w1.T + b1, m1)     # 129 -> 1000
    s2, m2 = lif(s1 @ w2.T + b2, m2)      # 1000 -> 1000
    s3, m3 = lif(s2 @ w3.T + b3, m3)      # 1000 -> 20
    s4, m4 = lif(s3 @ w4.T + b4, m4)      # 20 -> 10
with lif: reset = (m > 1); m' = 0.95*m + cur - reset; spk = (m' > 1)
Outputs: (s4, m4) per step -> each [81, 4096, 10].

Strategy (v3):
- Data parallel over 8 NeuronCores: 512 batch rows per core; weights
  replicated; no cross-device traffic.
- Hidden layers padded 1000 -> 1024 (8 tiles x 128): fp8 DoubleRow
  matmuls require full 32/64/128 weight-tile columns.
- Layers 1-3 run in a scaled membrane domain M = 2^12 * m so e4m3
  weight terms (w*2^12) are in-range, the sigmoid threshold trick is
  fp32-exact, and the LIF update is ONE fused DVE op per psum tile:
  M = beta*M + psum. The psum carries current, bias AND the
  -2^12*s_prev reset (via a -2048*I e5m2 diagonal, self-paired in
  DoubleRow: equal exponents make the 11-bit pair-combine exact).
- L2/L3 weights: 3-term e4m3 "band split" of w*2^12 run as DR matmuls
  at 0.5 PE cycles/col. The measured HW DR pair-combine is an 11-bit
  RNE adder, so term A holds only the >=4-magnitude entries (pair sums
  fit 11 bits: exact); terms B (<=4) and C (<=0.125) have negligible
  absolute combine error. Verified 0 output spike flips in a full-
  input numpy model of exactly this arithmetic.
- Spikes {0,1} live in e5m2 (fp8 matmul rhs), written by ACT as
  sigmoid(2^18*(M - 2^12)) which saturates to exactly 0.0/1.0.
- Biases ride always-on pad neurons: L1 pad units 1000..1007 get a
  +2^13 fp32r bias (combo matmul) so they fire every step and act as
  the ones-rows for L2's bias row; L2's pad unit 1000 is pulled up by
  3x8x224 e4m3 entries and provides L3's ones-row.
- L1 keeps the fp32r hi/lo split of x and w1 (24-bit; anything less
  flips thousands of output spikes) scaled by 2^12, 4 matmuls/tile,
  plus the DR diag.
- L4 is one packed fp32r matmul: lhsT rows [w4h|0|w4l|0|-I|b4h|b4l]
  vs rhs rows [s3|0|s3dup|0|s4_prev|ones]: bias, hi/lo split and the
  m4 reset ride one 213ns instruction. m4 stays unscaled (it is the
  output); s3/s4 are fp32r {0,1} written by Pool is_gt.
- Software pipeline per iteration i: L1(i+1)+ACT s1 (double-buffered);
  L2(i); L4(i-2); L3(i-1); ACT s2(i) last. The deferred L3/L4 keep the
  PE from ever waiting on a sigmoid.
"""

import os
import sys

import numpy as np
import ml_dtypes

for _p in ("/opt/trn_rl_repo", "/root/.axon_site/_ro/trn_rl_repo"):
    if os.path.isdir(_p) and _p not in sys.path:
        sys.path.insert(0, _p)

import concourse.bacc as bacc
import concourse.mybir as mybir
import concourse.tile as tile
from concourse.bass_utils import run_bass_kernel_spmd
from concourse.tile_rust import add_dep_helper

# Problem constants (hardcoded; kernel.py must be self-contained).
T = 81          # time steps
F = 129         # input features per step
H = 1000        # real hidden units (layers 1, 2)
HT = 128        # hidden tile rows (padded)
NH = 8          # number of hidden tiles
HP = NH * HT    # padded hidden 1024
H3 = 20         # layer-3 units
H3P = 32        # padded layer-3 psum partitions
H4 = 10         # output units
BATCH = 4096
NCORES = 8
B = BATCH // NCORES   # 512 batch rows per core
BETA = 0.95
XR = 2 * 128 + 5      # x_aug rows: xh[0:128], xl[0:128], 5 combo rows
MS = 4096.0           # membrane scale 2^12
SIG_SCALE = float(2.0 ** 18)
SIG_BIAS = float(-(2.0 ** 30))
K4 = 76               # L4 packed contraction rows
NONES = 8             # always-on L1 pad units 1000..1007

E4NP = ml_dtypes.float8_e4m3
E5NP = ml_dtypes.float8_e5m2

F32 = mybir.dt.float32
F32R = mybir.dt.float32r
FE4 = mybir.dt.float8e4
FE5 = mybir.dt.float8e5
AOP = mybir.AluOpType
DR = mybir.MatmulPerfMode.DoubleRow
SIGMOID = mybir.ActivationFunctionType.Sigmoid


def build_bass():
    # Bacc (not raw Bass): its compile() runs generate_event_semaphores /
    # move_matmul_waits_to_ldweights, required because TRN2 Matmult
    # instructions can carry at most one sync wait.
    nc = bacc.Bacc(trn_type="TRN2", target_bir_lowering=False)

    x_d = nc.dram_tensor("x_aug", [T, XR, B], F32R, kind="ExternalInput")
    w1h_d = nc.dram_tensor("w1h", [128, HP], F32R, kind="ExternalInput")
    w1l_d = nc.dram_tensor("w1l", [128, HP], F32R, kind="ExternalInput")
    w1c_d = nc.dram_tensor("w1c", [5, HP], F32R, kind="ExternalInput")
    w2t_d = [nc.dram_tensor(f"w2t{i}", [HT, NH, HP], FE4,
                            kind="ExternalInput") for i in range(3)]
    w3t_d = [nc.dram_tensor(f"w3t{i}", [HT, NH, H3P], FE4,
                            kind="ExternalInput") for i in range(3)]
    dg_d = nc.dram_tensor("dg", [HT, 2, HT], FE5, kind="ExternalInput")
    dg3_d = nc.dram_tensor("dg3", [H3, H3], F32R, kind="ExternalInput")
    w4c_d = nc.dram_tensor("w4c", [K4, H4], F32R, kind="ExternalInput")
    ones_d = nc.dram_tensor("ones2", [2, B], F32R, kind="ExternalInput")
    outs_d = nc.dram_tensor("out_s", [T, H4, B], F32, kind="ExternalOutput")
    outm_d = nc.dram_tensor("out_m", [T, H4, B], F32, kind="ExternalOutput")

    with tile.TileContext(nc) as tc:
        with (
            tc.tile_pool(name="pers", bufs=1) as pers,
            tc.tile_pool(name="xpool", bufs=3) as xpool,
            tc.tile_pool(name="ps1", bufs=3, space="PSUM") as ps1,
            tc.tile_pool(name="ps2", bufs=3, space="PSUM") as ps2,
            tc.tile_pool(name="ps3", bufs=1, space="PSUM") as ps3,
            tc.tile_pool(name="ps4", bufs=1, space="PSUM") as ps4,
        ):
            # ---- persistent SBUF tensors ----
            w1h = pers.tile([128, HP], F32R, tag="w1h")
            w1l = pers.tile([128, HP], F32R, tag="w1l")
            w1c = pers.tile([5, HP], F32R, tag="w1c")
            w2t = [pers.tile([HT, NH, HP], FE4, tag=f"w2t{i}",
                             name=f"w2t{i}") for i in range(3)]
            w3t = [pers.tile([HT, NH, H3P], FE4, tag=f"w3t{i}",
                             name=f"w3t{i}") for i in range(3)]
            dg = pers.tile([HT, 2, HT], FE5, tag="dg")      # -2048*I twice
            dg3 = pers.tile([H3, H3], F32R, tag="dg3")      # -4096*I
            w4c = pers.tile([K4, H4], F32R, tag="w4c")
            bias_t = pers.tile([128, 1], F32, tag="bias_t")  # -2^30

            M1 = pers.tile([HT, NH, B], F32, tag="M1")      # 2^12 * m1
            M2 = pers.tile([HT, NH, B], F32, tag="M2")
            M3 = pers.tile([H3P, B], F32, tag="M3")         # 2^12 * m3
            m4 = pers.tile([H4, B], F32, tag="m4")          # unscaled
            s1b = [pers.tile([HT, NH, B], FE5, tag=f"s1_{j}",
                             name=f"s1_{j}") for j in range(2)]  # dbl buf
            s2 = pers.tile([HT, NH, B], FE5, tag="s2")
            s3t = pers.tile([K4, B], F32R, tag="s3t")
            s3f = s3t[:].bitcast(F32)

            # ---- weight loads (layer-1 + x(0) + diag first: they gate
            # step 0; the bulk w2/w3 transfers follow) ----
            def load_x(t):
                xh = xpool.tile([128, B], F32R, tag="xh", name="xh")
                xl = xpool.tile([128, B], F32R, tag="xl", name="xl")
                xc = xpool.tile([5, B], F32R, tag="xc", name="xc")
                nc.sync.dma_start(xh[:], x_d[t, 0:128, :])
                nc.sync.dma_start(xl[:], x_d[t, 128:256, :])
                nc.sync.dma_start(xc[:], x_d[t, 256:261, :])
                return xh, xl, xc

            w1dmas = []
            for sb, dr_ in [(w1h, w1h_d), (w1l, w1l_d), (w1c, w1c_d),
                            (dg, dg_d)]:
                w1dmas.append(nc.sync.dma_start(sb[:], dr_[:]))
            x0 = load_x(0)
            wdmas = [nc.sync.dma_start(w4c[:], w4c_d[:]),
                     nc.sync.dma_start(dg3[:], dg3_d[:]),
                     nc.sync.dma_start(s3t[74:76, :], ones_d[:])]
            for i in range(3):
                wdmas.append(nc.sync.dma_start(w2t[i][:], w2t_d[i][:]))
                wdmas.append(nc.sync.dma_start(w3t[i][:], w3t_d[i][:]))

            # Matmult instructions can carry at most ONE sync wait in the
            # TRN2 ISA, so have PE nops absorb the weight-DMA waits before
            # any matmul.
            def absorb(dmas):
                nops = []
                for d in dmas:
                    nop = nc.tensor.nop(nofuse=True)
                    add_dep_helper(nop.ins, d.ins, sync=True,
                                   reason="absorb weight-DMA wait on PE")
                    nops.append(nop)
                return nops

            absorbers = absorb(w1dmas)

            # ---- state init ----
            nc.vector.memset(bias_t[:], SIG_BIAS)
            nc.vector.memset(M1[:], 0.0)
            nc.vector.memset(M2[:], 0.0)
            nc.gpsimd.memset(M3[:], 0.0)
            nc.gpsimd.memset(m4[:], 0.0)
            for j in range(2):
                nc.gpsimd.memset(s1b[j][:].bitcast(mybir.dt.uint8), 0)
            nc.gpsimd.memset(s2[:].bitcast(mybir.dt.uint8), 0)
            # s3t rows 0..73 zero; rows 74/75 are DMA'd ones.
            nc.gpsimd.memset(s3t[0:64].bitcast(F32), 0.0)
            nc.gpsimd.memset(s3t[64:74].bitcast(F32), 0.0)

            def sig(out_sl, in_sl, np_):
                return nc.scalar.activation(out_sl, in_sl, SIGMOID,
                                            bias=bias_t[0:np_],
                                            scale=SIG_SCALE)

            def l1_block(xh, xl, xc, sprev, first_abs=None):
                """Layer-1 psums + M1 update for one step.

                psum = 2^12*(w1 x + b1) - 2^12*s1_prev; M1 = b*M1 + psum.
                """
                first_mm = None
                for h in range(NH):
                    p1 = ps1.tile([HT, B], F32, tag="p1")
                    c0 = h * HT
                    mm = nc.tensor.matmul(p1[:], w1h[:, c0:c0 + HT], xh[:],
                                          start=True, stop=False)
                    if first_mm is None:
                        first_mm = mm
                        if first_abs:
                            for nop in first_abs:
                                add_dep_helper(
                                    mm.ins, nop.ins, sync=False,
                                    reason="absorbers before first matmul")
                    nc.tensor.matmul(p1[:], w1h[:, c0:c0 + HT], xl[:],
                                     start=False, stop=False)
                    nc.tensor.matmul(p1[:], w1l[:, c0:c0 + HT], xh[:],
                                     start=False, stop=False)
                    nc.tensor.matmul(p1[:], w1c[:, c0:c0 + HT], xc[:],
                                     start=False, stop=False)
                    nc.tensor.matmul(
                        p1[:], dg[:],
                        sprev[:, h:h + 1, :].to_broadcast((HT, 2, B)),
                        start=False, stop=True, perf_mode=DR)
                    nc.vector.scalar_tensor_tensor(
                        M1[:, h, :], M1[:, h, :], BETA, p1[:],
                        AOP.mult, AOP.add)

            # ---- prologue: step 0 layer-1 + s1(0) ----
            l1_block(*x0, sprev=s1b[1], first_abs=absorbers)
            sig(s1b[0][:], M1[:], HT)

            late_absorbers = absorb(wdmas)

            def l4_block(t):
                """Layer 4 for step t: packed matmul + LIF + output DMAs."""
                p4 = ps4.tile([H4, B], F32, tag="p4")
                nc.tensor.matmul(p4[:], w4c[:], s3t[:], start=True, stop=True)
                nc.vector.scalar_tensor_tensor(m4[:], m4[:], BETA, p4[:],
                                               AOP.mult, AOP.add)
                nc.sync.dma_start(outm_d[t], m4[:])
                nc.gpsimd.tensor_scalar(s3t[64:74, :], m4[:], 1.0, None,
                                        AOP.is_gt)
                nc.sync.dma_start(outs_d[t], s3f[64:74, :])

            def l3_block():
                """Layer 3: 8 DR band matmuls + fp32r reset diag.

                w3 runs 2-term (T_A + T_B, ~16-bit): layer 3 is insensitive
                (single 12-bit w3 already gives 0 output flips).
                """
                p3 = ps3.tile([H3P, B], F32, tag="p3")
                first = True
                for ti in range(2):
                    for g in range(4):
                        nc.tensor.matmul(
                            p3[:],
                            w3t[ti][:, 2 * g:2 * g + 2, :],
                            s2[:, 2 * g:2 * g + 2, :],
                            start=first, stop=False, perf_mode=DR)
                        first = False
                nc.tensor.matmul(p3[0:H3, :], dg3[:], s3t[0:H3, :],
                                 start=False, stop=True)
                nc.vector.scalar_tensor_tensor(M3[:], M3[:], BETA, p3[:],
                                               AOP.mult, AOP.add)
                nc.gpsimd.tensor_scalar(s3t[0:H3, :], M3[0:H3, :], MS, None,
                                        AOP.is_gt)
                # duplicate layer-3 spikes into rows 32..51 for the packed
                # hi+lo layer-4 matmul (DMA shifts partitions)
                nc.sync.dma_start(s3t[32:32 + H3, :], s3t[0:H3, :])

            # ---- main loop over steps ----
            for i in range(T):
                scur = s1b[i % 2]        # s1(i)
                snxt = s1b[(i + 1) % 2]  # s1(i+1)

                # layer 1 of step i+1 (+ its M1 update and sigmoid)
                if i < T - 1:
                    xs = load_x(i + 1)
                    l1_block(*xs, sprev=scur)
                    sig(snxt[:], M1[:], HT)

                # layer 2 of step i
                for h in range(NH):
                    p2 = ps2.tile([HT, B], F32, tag="p2")
                    c0 = h * HT
                    first_mm2 = None
                    for ti in range(3):
                        for g in range(4):
                            mm2 = nc.tensor.matmul(
                                p2[:],
                                w2t[ti][:, 2 * g:2 * g + 2, c0:c0 + HT],
                                scur[:, 2 * g:2 * g + 2, :],
                                start=(ti == 0 and g == 0), stop=False,
                                perf_mode=DR)
                            if first_mm2 is None:
                                first_mm2 = mm2
                                if i == 0 and h == 0:
                                    for nop in late_absorbers:
                                        add_dep_helper(
                                            mm2.ins, nop.ins, sync=False,
                                            reason="absorbers before L2")
                    nc.tensor.matmul(
                        p2[:], dg[:],
                        s2[:, h:h + 1, :].to_broadcast((HT, 2, B)),
                        start=False, stop=True, perf_mode=DR)
                    nc.vector.scalar_tensor_tensor(
                        M2[:, h, :], M2[:, h, :], BETA, p2[:],
                        AOP.mult, AOP.add)

                # layer 4 of step i-2 (deferred; s3(i-2) long ready)
                if i >= 2:
                    l4_block(i - 2)

                # layer 3 of step i-1 (deferred; s2(i-1) long ready; must
                # read s2 BEFORE this step's sigmoid overwrites it)
                if i >= 1:
                    l3_block()

                # s2(i) = sigmoid(M2): after L2 stts and after L3(i-1)
                # finished reading s2(i-1)
                sig(s2[:], M2[:], HT)

            # ---- epilogue ----
            l4_block(T - 2)
            l3_block()
            l4_block(T - 1)

    nc.compile()
    return nc


_CACHE = {}


def _get_nc():
    if "nc" not in _CACHE:
        _CACHE["nc"] = build_bass()
    return _CACHE["nc"]


def _rne12(a):
    """Round fp32 to 12 significand bits (the fp32r grid), RNE."""
    drop = np.uint64(12)
    u = np.ascontiguousarray(a, np.float32).view(np.uint32).astype(np.uint64)
    half = np.uint64(1 << 11)
    lsb = (u >> drop) & np.uint64(1)
    u2 = ((u + half - np.uint64(1) + lsb) >> drop << drop)
    return u2.astype(np.uint32).view(np.float32).reshape(a.shape)


def _hilo(a):
    hi = _rne12(np.asarray(a, np.float32))
    lo = _rne12((np.asarray(a, np.float64) - hi).astype(np.float32))
    return hi, lo


def _band3(w64):
    """3-term e4m3 band split of a 2^12-scaled fp64 array.

    t_A: entries of e4m3(w) with |v| >= 4 (DR pair sums exact at 11 bits)
    t_B: e4m3(residual)  (|v| <= 4: combine error <= 2^-9 absolute)
    t_C: e4m3(residual2) (|v| <= 0.125)
    """
    t1 = w64.astype(np.float32).astype(E4NP).astype(np.float64)
    ta = np.where(np.abs(t1) >= 4.0, t1, 0.0)
    r = w64 - ta
    tb = r.astype(np.float32).astype(E4NP)
    r2 = r - tb.astype(np.float64)
    tc = r2.astype(np.float32).astype(E4NP)
    return ta.astype(np.float32), tb.astype(np.float32), tc.astype(np.float32)


def _pack_terms(wT, bias, pullup_col=None):
    """[HT, NH, nout] e4m3 term tiles for a padded (1024 -> nout) layer.

    wT: [1000, nout] real weights (already col-padded); bias rides
    padded in-row 1000 (the first always-on pad neuron of the previous
    layer); optional pull-up column gets 3x8x224 from in-rows
    1000..1007.
    """
    nout = wT.shape[1]
    w64 = np.zeros((HP, nout), np.float64)
    w64[0:H] = np.asarray(wT, np.float64) * MS
    w64[H] += np.asarray(bias, np.float64) * MS
    terms = _band3(w64)
    out = []
    for t_ in terms:
        if pullup_col is not None:
            t_[H:H + NONES, pullup_col] = 224.0
        arr = np.zeros((HT, NH, nout), E4NP)
        for k in range(NH):
            arr[:, k, :] = t_[k * HT:(k + 1) * HT, :]
        out.append(arr)
    return out


def _prep_inputs(x, w1, b1, w2, b2, w3, b3, w4, b4):
    x = np.ascontiguousarray(x, np.float32)
    # xs[t, f, b_global]; step t of the reference reads x[:, f*T + t]
    xt = np.ascontiguousarray(
        np.transpose(x.reshape(BATCH, F, T), (2, 1, 0)))   # [T, F, BATCH]
    xth, xtl = _hilo(xt)

    # L1 weights: [129, 1024] padded, scaled by 2^12.
    w1P = np.zeros((F, HP), np.float64)
    w1P[:, 0:H] = np.asarray(w1, np.float64).T * MS
    w1h, w1l = _hilo(w1P[0:128].astype(np.float32))
    whL, wlL = _hilo(w1P[128].astype(np.float32))
    b1r = np.zeros(HP, np.float64)
    b1r[0:H] = np.asarray(b1, np.float64) * MS
    b1r[H:H + NONES] = 2.0 ** 13        # always-on pad-neuron pull-up
    b1h, b1l = _hilo(b1r.astype(np.float32))
    w1c = np.stack([whL, whL, wlL, b1h, b1l])              # [5, 1024]

    # L2: cols padded to 1024; pull-up col = padded unit 1000.
    w2full = np.zeros((H, HP), np.float32)
    w2full[:, 0:H] = np.asarray(w2, np.float32).T
    b2full = np.zeros(HP, np.float32)
    b2full[0:H] = np.asarray(b2, np.float32)
    w2terms = _pack_terms(w2full, b2full, pullup_col=H)
    w3full = np.zeros((H, H3P), np.float32)
    w3full[:, 0:H3] = np.asarray(w3, np.float32).T
    b3full = np.zeros(H3P, np.float32)
    b3full[0:H3] = np.asarray(b3, np.float32)
    w3terms = _pack_terms(w3full, b3full)

    dgv = np.zeros((HT, 2, HT), np.float32)
    for i in range(HT):
        dgv[i, 0, i] = -2048.0
        dgv[i, 1, i] = -2048.0
    dg = dgv.astype(E5NP)
    dg3 = (-MS * np.eye(H3, dtype=np.float32))

    # L4 packed lhsT: rows 0..19 w4h, 32..51 w4l, 64..73 -I (s4 reset),
    # 74 b4h, 75 b4l; rest zero.
    w4T = np.asarray(w4, np.float32).T                     # [20, 10]
    w4h, w4l = _hilo(w4T)
    b4h, b4l = _hilo(np.asarray(b4, np.float32))
    w4c = np.zeros((K4, H4), np.float32)
    w4c[0:H3] = w4h
    w4c[32:32 + H3] = w4l
    w4c[64:64 + H4] = -np.eye(H4, dtype=np.float32)
    w4c[74] = b4h
    w4c[75] = b4l

    ones2 = np.ones((2, B), np.float32)

    in_maps = []
    for c in range(NCORES):
        xc = np.empty((T, XR, B), np.float32)
        xc[:, 0:128, :] = xth[:, 0:128, c * B:(c + 1) * B]
        xc[:, 128:256, :] = xtl[:, 0:128, c * B:(c + 1) * B]
        xc[:, 256, :] = xth[:, 128, c * B:(c + 1) * B]
        xc[:, 257, :] = xtl[:, 128, c * B:(c + 1) * B]
        xc[:, 258, :] = xth[:, 128, c * B:(c + 1) * B]
        xc[:, 259, :] = 1.0
        xc[:, 260, :] = 1.0
        m = {"x_aug": xc, "w1h": w1h, "w1l": w1l, "w1c": w1c,
             "dg": dg, "dg3": dg3, "w4c": w4c, "ones2": ones2}
        for i in range(3):
            m[f"w2t{i}"] = w2terms[i]
            m[f"w3t{i}"] = w3terms[i]
        in_maps.append(m)
    return in_maps


def _gather(results):
    spk = np.concatenate(
        [np.transpose(r["out_s"], (0, 2, 1)) for r in results], axis=1)
    mem = np.concatenate(
        [np.transpose(r["out_m"], (0, 2, 1)) for r in results], axis=1)
    return spk, mem


def kernel(x, w1, b1, w2, b2, w3, b3, w4, b4, _trace=False, _trace_kwargs=None):
    x, w1, b1, w2, b2, w3, b3, w4, b4 = (
        np.asarray(a, dtype=np.float32)
        for a in (x, w1, b1, w2, b2, w3, b3, w4, b4))
    nc = _get_nc()
    in_maps = _prep_inputs(x, w1, b1, w2, b2, w3, b3, w4, b4)
    res = run_bass_kernel_spmd(
        nc, in_maps, core_ids=list(range(NCORES)),
        trace=_trace, **(_trace_kwargs or {}))
    out = _gather(res.results)
    if _trace:
        return out, res
    return out
